# revision 1
# baseline (speedup 1.0000x reference)
"""DeepseekV2 decoder layer on 8 TRN2 NeuronCores (Bass/Tile).

Sharding: TP over heads (2/core) for q/kv_b/attention/o_proj, kv_a sharded
over tokens (256/core) + AllGather, TP over INTER (1024/core) for the MLP.
Chunked AllReduce after o_proj and chunked ReduceScatter after down_proj,
overlapped with compute.

Internal layout is feature-major ("transposed"): activations live as
[feature, token] so every matmul output feeds the next as `rhs` without any
on-device transpose. RoPE pair-swaps, RMSNorm weight folding, the softmax
scaling, and cos/sin tables are all folded into host-side weight prep.

All DRAM tensors are pre-tiled on the host to [128, ...] partition-major
layout so every load/store is a single large dma_start (128 fat
descriptors) instead of hundreds of small ones.
"""

import numpy as np
import ml_dtypes

import concourse.bass as bass
import concourse.mybir as mybir
import concourse.tile as tile
from concourse import bacc
from concourse.bass_utils import run_bass_kernel_spmd

BF = ml_dtypes.bfloat16

B, S, HID = 2, 1024, 2048
T = B * S                      # 2048 tokens
H = 16
DN, DR = 128, 64
DQK = DN + DR
DV = 128
KVR = 512
INTER = 8192
EPS = 1e-6
ROPE_BASE = 10000.0
SCALING = DQK ** -0.5

NC_N = 8
HPC = H // NC_N                # 2 heads per core
FPC = INTER // NC_N            # 1024 inter per core
P = 128
HCH = HID // P                 # 16 hid chunks
TT = 4                         # token chunks of 512
TW = T // TT                   # 512
TO = T // NC_N                 # 256 own tokens for kv_a
KT = S // P                    # 8 k-tiles of 128 per batch
QT = S // TW                   # 2 q-chunks of 512 per batch
KC = KVR // P                  # 4 kv-lora chunks
NEG = -30000.0

f32 = mybir.dt.float32
bf16 = mybir.dt.bfloat16
ADD = mybir.AluOpType.add
MUL = mybir.AluOpType.mult
BYP = mybir.AluOpType.bypass
AF = mybir.ActivationFunctionType

_CACHE = {}


def _build():
    nc = bacc.Bacc("TRN2", target_bir_lowering=False, debug=False, num_devices=NC_N)
    dp = lambda n, sh, dt: nc.dram_tensor(n, sh, dt, kind="ExternalInput")
    htb = dp("htb", [P, TT, HCH, TW], bf16)     # hidden^T, chunk-tiled
    hto = dp("hto", [P, HCH, TO], bf16)         # own-token slice of hidden^T
    wq = dp("wq", [P, HCH, HPC * DQK], bf16)    # [h0n,h1n,h0x1,h0x2,h1x1,h1x2]
    wkva = dp("wkva", [P, HCH, KVR + DR], bf16)  # kv cols + pe (pair-split)
    wkvb = dp("wkvb", [P, KC, HPC * (DN + DV)], bf16)
    wo = dp("wo", [P, HPC, HID], bf16)
    wg = dp("wg", [P, HCH, FPC], bf16)
    wu = dp("wu", [P, HCH, FPC], bf16)
    wd = dp("wd", [P, FPC // P, HID], bf16)
    cosf = dp("cosf", [P, T], bf16)
    sinf = dp("sinf", [P, T], bf16)
    masks = dp("masks", [P, 4, TW], f32)
    out = nc.dram_tensor("o", [16, TT * HCH * TW], bf16, kind="ExternalOutput")
    rg = [list(range(NC_N))]

    with tile.TileContext(nc) as tc:
        with tc.tile_pool(name="const", bufs=1) as cpool, \
             tc.tile_pool(name="dram", bufs=1, space="DRAM") as dram, \
             tc.tile_pool(name="mlpw", bufs=1) as mlpw:
            ones_col = cpool.tile([P, 1], bf16)
            nc.vector.memset(ones_col[:], 1.0)
            ones_row = cpool.tile([1, P], bf16)
            nc.vector.memset(ones_row[:], 1.0)
            epsb = cpool.tile([1, 1], f32)
            nc.vector.memset(epsb[:], EPS)

            ag_in = dram.tile([P, KC * TO], bf16, name="ag_in")
            ag_out = dram.tile([NC_N * P, KC * TO], bf16, addr_space="Shared",
                               name="ag_out")
            # o_proj reduction as RS + AG (4x less wire than mesh AllReduce)
            ar_in = [dram.tile([P, HCH, TW], bf16, name=f"ar_in{t}")
                     for t in range(TT)]
            o_rs = [dram.tile([16, HCH * TW], bf16, name=f"o_rs{t}")
                    for t in range(TT)]
            o_ag = [dram.tile([P, HCH, TW], bf16, addr_space="Shared",
                              name=f"o_ag{t}") for t in range(TT)]
            # chunks 0-2 reduce-scatter 2MB; chunk 3 in two 1MB halves (tail)
            rs_in = [dram.tile([P, HCH, TW], bf16, name=f"rs_in{t}")
                     for t in range(TT - 1)]
            rs_in += [dram.tile([P, HCH // 2, TW], bf16, name=f"rs_in3{i}")
                      for i in range(2)]
            rs_out = [dram.tile([16, HCH * TW], bf16, name=f"rs_out{t}")
                      for t in range(TT - 1)]
            rs_out += [dram.tile([16, HCH // 2 * TW], bf16, name=f"rs_out3{i}")
                       for i in range(2)]

            # ============ Phase A: projections + attention ============
            with tc.tile_pool(name="akeep", bufs=1) as akeep, \
                 tc.tile_pool(name="awrk", bufs=2) as awrk, \
                 tc.tile_pool(name="arow", bufs=2) as arow, \
                 tc.tile_pool(name="aps", bufs=1, space="PSUM") as aps:

                # survives A1 -> A2
                qsb = akeep.tile([P, 3, T], bf16)          # 12K
                kpe2 = akeep.tile([P, T], bf16)            # 4K (dup'd rope rows)

                # ---- A1: input norm + q/kv_a projections ----
                with tc.tile_pool(name="a1", bufs=1) as a1:
                    hto_sb = a1.tile([P, HCH, TO], bf16)
                    nc.scalar.dma_start(hto_sb[:], hto.ap())
                    wkva_sb = a1.tile([P, HCH, KVR + DR], bf16)
                    nc.scalar.dma_start(wkva_sb[:], wkva.ap())
                    wq_sb = a1.tile([P, HCH, HPC * DQK], bf16)
                    nc.scalar.dma_start(wq_sb[:], wq.ap())

                    wkvb_sb = akeep.tile([P, KC, HPC * (DN + DV)], bf16,
                                         name="wkvb_sb")
                    nc.scalar.dma_start(wkvb_sb[:], wkvb.ap())
                    wo_sb = akeep.tile([P, HPC, HID], bf16, name="wo_sb")
                    nc.scalar.dma_start(wo_sb[:], wo.ap())

                    # -- input rmsnorm scale + q proj + k_pe, per token chunk --
                    for t in range(TT):
                        ht_t = a1.tile([P, HCH, TW], bf16, tag="ht", bufs=2,
                                       name="ht_t")
                        nc.sync.dma_start(ht_t[:], htb.ap()[:, t, :, :])
                        # sum of squares: scalar squares, quad-group adds on
                        # vector, 4 matmul reduces
                        ssp = aps.tile([1, TW], f32, tag="ss", bufs=2, name="ssp")
                        for g in range(4):
                            sqg = awrk.tile([P, TW], bf16, tag="sqg", name="sqg")
                            for k in range(4):
                                o = 4 * g + k
                                if k == 0:
                                    nc.scalar.square(sqg[:], ht_t[:, o, :])
                                else:
                                    sq = awrk.tile([P, TW], bf16, tag="sq",
                                                   name="sq")
                                    nc.scalar.square(sq[:], ht_t[:, o, :])
                                    nc.vector.tensor_tensor(sqg[:], sqg[:], sq[:],
                                                            ADD)
                            nc.tensor.matmul(ssp[:], ones_col[:], sqg[:],
                                             start=(g == 0), stop=(g == 3))
                        lg = arow.tile([1, TW], f32, tag="srow", name="lg")
                        nc.scalar.activation(lg[:], ssp[:], AF.Ln,
                                             bias=epsb[:], scale=1.0 / HID)
                        rb = arow.tile([1, TW], bf16, tag="rb", name="rb")
                        nc.scalar.activation(rb[:], lg[:], AF.Exp, scale=-0.5)
                        bcp = aps.tile([P, TW], f32, tag="big", bufs=2, name="bcp")
                        nc.tensor.matmul(bcp[:], ones_row[:], rb[:],
                                         start=True, stop=True)
                        bc1 = a1.tile([P, TW], f32, tag="bc1", bufs=1, name="bc1")
                        nc.vector.tensor_copy(out=bc1[:], in_=bcp[:])

                        # q + k_pe projections: copy raw to SBUF immediately
                        # (frees PSUM), scale by r1 in place once bc1 is ready
                        for f in range(3):
                            qp = aps.tile([P, TW], f32, tag="big", bufs=2, name="qp")
                            for o in range(HCH):
                                nc.tensor.matmul(qp[:], wq_sb[:, o, f * P:(f + 1) * P],
                                                 ht_t[:, o, :],
                                                 start=(o == 0), stop=(o == HCH - 1))
                            nc.vector.tensor_copy(
                                out=qsb[:, f, t * TW:(t + 1) * TW], in_=qp[:])
                        kp2 = aps.tile([P, TW], f32, tag="big", bufs=2, name="kp2")
                        for o in range(HCH):
                            nc.tensor.matmul(kp2[:DR, :],
                                             wkva_sb[:, o, KVR:KVR + DR],
                                             ht_t[:, o, :],
                                             start=(o == 0), stop=(o == HCH - 1))
                        nc.vector.tensor_copy(out=kpe2[:DR, t * TW:(t + 1) * TW],
                                              in_=kp2[:DR, :])
                        for f in range(3):
                            nc.vector.tensor_tensor(qsb[:, f, t * TW:(t + 1) * TW],
                                                    qsb[:, f, t * TW:(t + 1) * TW],
                                                    bc1[:], MUL)
                        nc.vector.tensor_tensor(kpe2[:DR, t * TW:(t + 1) * TW],
                                                kpe2[:DR, t * TW:(t + 1) * TW],
                                                bc1[:DR, :], MUL)
                        # duplicate rope rows for the second attention head
                        nc.sync.dma_start(kpe2[DR:, t * TW:(t + 1) * TW],
                                          kpe2[:DR, t * TW:(t + 1) * TW])
                        if t == 1:
                            # -- kv_a for OWN 256 tokens (sharded), then AllGather --
                            lat_own = a1.tile([P, KC, TO], bf16)
                            ss2p = aps.tile([1, TW], f32, tag="ss", bufs=2, name="ss2p")
                            sqg2 = awrk.tile([P, TO], bf16, tag="sq", name="sqg2")
                            for f in range(KC):
                                lp = aps.tile([P, TO], f32, tag="big", bufs=2, name="lp")
                                for o in range(HCH):
                                    nc.tensor.matmul(lp[:], wkva_sb[:, o, f * P:(f + 1) * P],
                                                     hto_sb[:, o, :],
                                                     start=(o == 0), stop=(o == HCH - 1))
                                nc.vector.tensor_copy(out=lat_own[:, f, :], in_=lp[:])
                                if f == 0:
                                    nc.scalar.square(sqg2[:], lat_own[:, f, :])
                                else:
                                    sq2 = awrk.tile([P, TO], bf16, tag="sq", name="sq2")
                                    nc.scalar.square(sq2[:], lat_own[:, f, :])
                                    nc.vector.tensor_tensor(sqg2[:], sqg2[:], sq2[:], ADD)
                            nc.tensor.matmul(ss2p[:, :TO], ones_col[:], sqg2[:],
                                             start=True, stop=True)
                            lg2 = arow.tile([1, TO], f32, tag="srow", name="lg2")
                            nc.scalar.activation(lg2[:], ss2p[:, :TO], AF.Ln,
                                                 bias=epsb[:], scale=1.0 / KVR)
                            rb2 = arow.tile([1, TO], bf16, tag="rb", name="rb2")
                            nc.scalar.activation(rb2[:], lg2[:], AF.Exp, scale=-0.5)
                            bcp2 = aps.tile([P, TW], f32, tag="att", bufs=2, name="bcp2")
                            nc.tensor.matmul(bcp2[:, :TO], ones_row[:], rb2[:],
                                             start=True, stop=True)
                            bc2 = a1.tile([P, TO], f32, name="bc2")
                            nc.vector.tensor_copy(out=bc2[:], in_=bcp2[:, :TO])
                            for f in range(KC):
                                nc.vector.tensor_tensor(lat_own[:, f, :], lat_own[:, f, :],
                                                        bc2[:], MUL)
                            nc.sync.dma_start(ag_in[:], lat_own[:])
                            nc.gpsimd.collective_compute(
                                "AllGather", BYP, ins=[ag_in[:].opt()],
                                outs=[ag_out[:].opt()], replica_groups=rg)


                    # readback of gathered kv_a (after ht loads in ring order;
                    # kv_b matmuls overlap the tail of A1)
                    kva2 = akeep.tile([P, NC_N, KC, TO], bf16, name="kva2")
                    for r in range(NC_N):
                        nc.gpsimd.dma_start(kva2[:, r, :, :],
                                            ag_out[r * P:(r + 1) * P, :])

                # ---- A2: rope, kv_b, attention, o_proj (+AR) ----
                with tc.tile_pool(name="a2", bufs=1) as a2:
                    cs = a2.tile([P, T], bf16)
                    nc.scalar.dma_start(cs[:], cosf.ap())
                    sn = a2.tile([P, T], bf16)
                    nc.scalar.dma_start(sn[:], sinf.ap())
                    msk = a2.tile([P, 4, TW], f32)
                    nc.scalar.dma_start(msk[:], masks.ap())
                    # prefetch the big MLP weights early (consumed in phase B)
                    wg_sb = mlpw.tile([P, HCH, FPC], bf16)       # 32K
                    nc.scalar.dma_start(wg_sb[:], wg.ap())
                    wu_sb = mlpw.tile([P, HCH, FPC], bf16)       # 32K
                    nc.scalar.dma_start(wu_sb[:], wu.ap())

                    # kv_b: k_nope (transposed out) + v (natural out)
                    knope = a2.tile([P, HPC, T], bf16)
                    for h in range(HPC):
                        for t2 in range(NC_N):
                            kp = aps.tile([P, TO], f32, tag="big", bufs=2, name="kp")
                            for c in range(KC):
                                nc.tensor.matmul(kp[:],
                                                 wkvb_sb[:, c, h * P:(h + 1) * P],
                                                 kva2[:, t2, c, :],
                                                 start=(c == 0), stop=(c == KC - 1))
                            nc.vector.tensor_copy(
                                out=knope[:, h, t2 * TO:(t2 + 1) * TO], in_=kp[:])
                    vnat = a2.tile([P, T // P, HPC * DV], bf16)
                    for to in range(T // P):
                        vp = aps.tile([P, HPC * DV], f32, tag="vp", bufs=1, name="vp")
                        for c in range(KC):
                            nc.tensor.matmul(vp[:],
                                             kva2[:, to // 2, c,
                                                  (to % 2) * P:(to % 2 + 1) * P],
                                             wkvb_sb[:, c, HPC * DN:],
                                             start=(c == 0), stop=(c == KC - 1))
                        nc.vector.tensor_copy(out=vnat[:, to, :], in_=vp[:])

                    # rope in place: qsb[:,2,:] rows are [h0x1,h0x2,h1x1,h1x2],
                    # kpe2 rows are [x1,x2,x1,x2]; cs=[c,c,c,c], sn=[-s,s,-s,s]
                    for src in (qsb[:, 2, :], kpe2[:]):
                        swp = a2.tile([P, T], bf16, tag="swp", bufs=2, name="swp")
                        for g in range(4):
                            half = 32 if g % 2 == 0 else -32
                            nc.sync.dma_start(swp[g * 32:(g + 1) * 32, :],
                                              src[g * 32 + half:(g + 1) * 32 + half, :])
                        rtmp = a2.tile([P, T], bf16, tag="rtmp", bufs=2, name="rtmp")
                        nc.vector.tensor_tensor(rtmp[:], src, cs[:], MUL)
                        nc.vector.tensor_tensor(src, swp[:], sn[:], MUL)
                        nc.vector.tensor_tensor(src, src, rtmp[:], ADD)

                    # attention (scores transposed: [k, q]) + o_proj partial + AR
                    for b in range(B):
                        for qt in range(QT):
                            tt = b * QT + qt
                            qc0 = b * S + qt * TW
                            nkt = 4 * qt + 4
                            attn_t = a2.tile([P, HPC, TW], bf16, tag="attn",
                                             bufs=2, name="attn_t")
                            for h in range(HPC):
                                dnp = aps.tile([1, TW], f32, tag="den", bufs=1,
                                               name="dnp")
                                atp = aps.tile([P, TW], f32, tag="att", bufs=2,
                                               name="atp")
                                exs = [None] * nkt

                                def consume(kt):
                                    nc.tensor.matmul(dnp[:], ones_col[:], exs[kt][:],
                                                     start=(kt == 0),
                                                     stop=(kt == nkt - 1))
                                    nc.tensor.matmul(atp[:],
                                                     vnat[:, b * KT + kt,
                                                          h * DV:(h + 1) * DV],
                                                     exs[kt][:],
                                                     start=(kt == 0),
                                                     stop=(kt == nkt - 1))

                                for kt in range(nkt):
                                    kc0 = b * S + kt * P
                                    scp = aps.tile([P, TW], f32, tag="big", bufs=2,
                                                   name="scp")
                                    nc.tensor.matmul(scp[:],
                                                     knope[:, h, kc0:kc0 + P],
                                                     qsb[:, h, qc0:qc0 + TW],
                                                     start=True, stop=False)
                                    nc.tensor.matmul(
                                        scp[:],
                                        kpe2[h * DR:(h + 1) * DR, kc0:kc0 + P],
                                        qsb[h * DR:(h + 1) * DR, 2, qc0:qc0 + TW],
                                        start=False, stop=True)
                                    ex = awrk.tile([P, TW], bf16, tag="ex", bufs=4,
                                                   name="ex")
                                    j = kt - 4 * qt
                                    if j >= 0:
                                        mtmp = awrk.tile([P, TW], f32, tag="mt",
                                                         name="mtmp")
                                        nc.vector.tensor_tensor(mtmp[:], scp[:],
                                                                msk[:, j, :], ADD)
                                        nc.scalar.activation(ex[:], mtmp[:], AF.Exp)
                                    else:
                                        nc.scalar.activation(ex[:], scp[:], AF.Exp)
                                    exs[kt] = ex
                                    if kt >= 2:
                                        consume(kt - 2)
                                consume(max(nkt - 2, 0))
                                if nkt > 1:
                                    consume(nkt - 1)
                                dlg = arow.tile([1, TW], f32, tag="srow",
                                                name="dlg")
                                nc.scalar.activation(dlg[:], dnp[:], AF.Ln)
                                drow = arow.tile([1, TW], bf16, tag="rb",
                                                 name="drow")
                                nc.scalar.activation(drow[:], dlg[:], AF.Exp,
                                                     scale=-1.0)
                                dbp = aps.tile([P, TW], f32, tag="big", bufs=2,
                                               name="dbp")
                                nc.tensor.matmul(dbp[:], ones_row[:], drow[:],
                                                 start=True, stop=True)
                                dbc = awrk.tile([P, TW], f32, tag="mt", name="dbc")
                                nc.vector.tensor_copy(out=dbc[:], in_=dbp[:])
                                nc.vector.tensor_tensor(
                                    attn_t[:, h, :], atp[:], dbc[:], MUL)
                            # o_proj partial for this token chunk
                            oall = a2.tile([P, HCH, TW], bf16, tag="oall", bufs=1,
                                           name="oall")
                            for ho in range(HCH):
                                op = aps.tile([P, TW], f32, tag="big", bufs=2,
                                              name="op")
                                for h in range(HPC):
                                    nc.tensor.matmul(op[:],
                                                     wo_sb[:, h, ho * P:(ho + 1) * P],
                                                     attn_t[:, h, :],
                                                     start=(h == 0),
                                                     stop=(h == HPC - 1))
                                nc.vector.tensor_copy(out=oall[:, ho, :], in_=op[:])
                            nc.sync.dma_start(ar_in[tt][:], oall[:])
                            nc.gpsimd.collective_compute(
                                "ReduceScatter", ADD, ins=[ar_in[tt][:].opt()],
                                outs=[o_rs[tt][:].opt()], replica_groups=rg)
                            nc.gpsimd.collective_compute(
                                "AllGather", BYP, ins=[o_rs[tt][:].opt()],
                                outs=[o_ag[tt][:].opt()], replica_groups=rg)

            # ============ Phase B: residual + norm + MLP ============
            with tc.tile_pool(name="bbig", bufs=1) as bbig, \
                 tc.tile_pool(name="bwrk", bufs=2) as bwrk, \
                 tc.tile_pool(name="brow", bufs=1) as brow, \
                 tc.tile_pool(name="bps", bufs=1, space="PSUM") as bps:

                wd_sb = bbig.tile([P, FPC // P, HID], bf16)  # 32K
                nc.scalar.dma_start(wd_sb[:], wd.ap())

                def load_chunk(t):
                    xt = bbig.tile([P, HCH, TW], bf16, name=f"x{t}", tag="x",
                                   bufs=2)
                    nc.sync.dma_start(xt[:], htb.ap()[:, t, :, :])
                    art = bbig.tile([P, HCH, TW], bf16, name=f"ar{t}", tag="ar",
                                    bufs=1)
                    nc.sync.dma_start(art[:], o_ag[t][:])
                    return xt, art

                pend = load_chunk(0)
                for t in range(TT):
                    # x = hidden + attn_out; later x/8 in place
                    x, arall = pend
                    ssp3 = bps.tile([1, TW], f32, tag="ss", bufs=1, name="ssp3")
                    for g in range(4):
                        sqg3 = bwrk.tile([P, TW], bf16, tag="sqg3", bufs=2,
                                         name="sqg3")
                        for k in range(4):
                            o = 4 * g + k
                            nc.vector.tensor_tensor(x[:, o, :], x[:, o, :],
                                                    arall[:, o, :], ADD)
                            if k == 0:
                                nc.scalar.square(sqg3[:], x[:, o, :])
                            else:
                                sq3 = bwrk.tile([P, TW], bf16, tag="sq3", bufs=2,
                                                name="sq3")
                                nc.scalar.square(sq3[:], x[:, o, :])
                                nc.vector.tensor_tensor(sqg3[:], sqg3[:], sq3[:],
                                                        ADD)
                        nc.tensor.matmul(ssp3[:], ones_col[:], sqg3[:],
                                         start=(g == 0), stop=(g == 3))
                    lg3 = brow.tile([1, TW], f32, tag="lg3", name="lg3")
                    nc.scalar.activation(lg3[:], ssp3[:], AF.Ln,
                                         bias=epsb[:], scale=1.0 / HID)
                    rb3 = brow.tile([1, TW], bf16, tag="rb3", name="rb3")
                    nc.scalar.activation(rb3[:], lg3[:], AF.Exp, scale=-0.5)
                    bcp3 = bps.tile([P, TW], f32, tag="gu", bufs=4, name="bcp3")
                    nc.tensor.matmul(bcp3[:], ones_row[:], rb3[:], start=True,
                                     stop=True)
                    bc3 = bwrk.tile([P, TW], f32, tag="bc3", bufs=1, name="bc3")
                    nc.vector.tensor_copy(out=bc3[:], in_=bcp3[:])
                    h2 = bbig.tile([P, HCH, TW], bf16, name="h2", tag="h2", bufs=1)
                    for o in range(HCH):
                        nc.vector.tensor_tensor(h2[:, o, :], x[:, o, :], bc3[:], MUL)
                    # x -> x/8 in place (folded residual for ReduceScatter)
                    nc.vector.tensor_scalar_mul(x[:, :, :], x[:, :, :], 0.125)
                    # issue next chunk's loads BEFORE this chunk's stores so
                    # they are not stuck behind them in the HWDGE ring
                    if t + 1 < TT:
                        pend = load_chunk(t + 1)

                    # gate/up/silu
                    act = bbig.tile([P, FPC // P, TW], bf16, name="act", tag="act",
                                    bufs=1)
                    for fi in range(FPC // P):
                        gp = bps.tile([P, TW], f32, tag="gu", bufs=4, name="gp")
                        for o in range(HCH):
                            nc.tensor.matmul(gp[:], wg_sb[:, o, fi * P:(fi + 1) * P],
                                             h2[:, o, :],
                                             start=(o == 0), stop=(o == HCH - 1))
                        up = bps.tile([P, TW], f32, tag="gu", bufs=4, name="up")
                        for o in range(HCH):
                            nc.tensor.matmul(up[:], wu_sb[:, o, fi * P:(fi + 1) * P],
                                             h2[:, o, :],
                                             start=(o == 0), stop=(o == HCH - 1))
                        gs = bwrk.tile([P, TW], f32, tag="gs", bufs=2, name="gs")
                        nc.scalar.activation(gs[:], gp[:], AF.Silu)
                        nc.vector.tensor_tensor(act[:, fi, :], up[:], gs[:], MUL)

                    # down projection partial (+x/8) + RS
                    # last chunk reduces in two 1MB halves to shrink the tail
                    dall = bbig.tile([P, HCH, TW], bf16, name="dall", tag="dall",
                                     bufs=1)
                    for ho in range(HCH):
                        dpp = bps.tile([P, TW], f32, tag="d", bufs=2, name="dpp")
                        for c in range(FPC // P):
                            nc.tensor.matmul(dpp[:], wd_sb[:, c, ho * P:(ho + 1) * P],
                                             act[:, c, :],
                                             start=(c == 0), stop=(c == FPC // P - 1))
                        nc.vector.tensor_tensor(dall[:, ho, :], dpp[:], x[:, ho, :],
                                                ADD)
                        if t == TT - 1 and ho == HCH // 2 - 1:
                            nc.sync.dma_start(rs_in[3][:], dall[:, :HCH // 2, :])
                            nc.gpsimd.collective_compute(
                                "ReduceScatter", ADD, ins=[rs_in[3][:].opt()],
                                outs=[rs_out[3][:].opt()], replica_groups=rg)
                            nc.sync.dma_start(
                                out.ap()[:, 3 * HCH * TW:
                                         3 * HCH * TW + HCH // 2 * TW],
                                rs_out[3][:])
                    if t < TT - 1:
                        nc.sync.dma_start(rs_in[t][:], dall[:])
                        nc.gpsimd.collective_compute(
                            "ReduceScatter", ADD, ins=[rs_in[t][:].opt()],
                            outs=[rs_out[t][:].opt()], replica_groups=rg)
                        nc.sync.dma_start(
                            out.ap()[:, t * HCH * TW:(t + 1) * HCH * TW],
                            rs_out[t][:])
                    else:
                        nc.sync.dma_start(rs_in[4][:], dall[:, HCH // 2:, :])
                        nc.gpsimd.collective_compute(
                            "ReduceScatter", ADD, ins=[rs_in[4][:].opt()],
                            outs=[rs_out[4][:].opt()], replica_groups=rg)
                        nc.sync.dma_start(
                            out.ap()[:, 3 * HCH * TW + HCH // 2 * TW:],
                            rs_out[4][:])
    nc.compile()
    return nc


def _prep(hidden_states, positions, w_in_ln, w_q, w_kv_a, w_kv_a_ln,
          w_kv_b, w_o, w_post_ln, w_gate, w_up, w_down):
    hT = np.ascontiguousarray(
        np.asarray(hidden_states, np.float32).reshape(T, HID).T)
    hTt = hT.reshape(HCH, P, T).transpose(1, 0, 2)          # [128, 16, T]
    htb = np.ascontiguousarray(
        hTt.reshape(P, HCH, TT, TW).transpose(0, 2, 1, 3)).astype(BF)

    pos = np.asarray(positions).reshape(-1).astype(np.float64)
    inv = ROPE_BASE ** (-np.arange(0, DR, 2, dtype=np.float64) / DR)
    fr = pos[:, None] * inv[None, :]                      # [T, 32]
    c32 = np.cos(fr).T.astype(np.float32)                 # [32, T]
    s32 = np.sin(fr).T.astype(np.float32)
    cosf = np.concatenate([c32] * 4, 0)
    sinf = np.concatenate([-s32, s32, -s32, s32], 0)

    r = np.arange(P)[:, None]
    c = np.arange(TW)[None, :]
    masks = np.stack([np.where(c >= r + j * P, 0.0, NEG) for j in range(4)],
                     1).astype(np.float32)                # [128, 4, 512]

    def tilemaj(a, chunks):
        # [rows, cols] -> [128, chunks, cols], partition-major
        return np.ascontiguousarray(
            a.reshape(chunks, P, -1).transpose(1, 0, 2)).astype(BF)

    w_in_ln = np.asarray(w_in_ln, np.float32)
    wqf = (np.asarray(w_q, np.float32) * w_in_ln[:, None] * SCALING
           ).reshape(HID, H, DQK)
    wkvaf = np.asarray(w_kv_a, np.float32) * w_in_ln[:, None]
    kpe_w = wkvaf[:, KVR:]
    pe_pair = np.concatenate([kpe_w[:, 0::2], kpe_w[:, 1::2]], 1)
    wkva_p = np.concatenate([wkvaf[:, :KVR], pe_pair], 1)
    wkvbf = (np.asarray(w_kv_b, np.float32)
             * np.asarray(w_kv_a_ln, np.float32)[:, None]).reshape(KVR, H, DN + DV)
    w_post_ln = np.asarray(w_post_ln, np.float32)
    wgf = np.asarray(w_gate, np.float32) * w_post_ln[:, None]
    wuf = np.asarray(w_up, np.float32) * w_post_ln[:, None]
    wdf = np.asarray(w_down, np.float32)
    wof = np.asarray(w_o, np.float32).reshape(H, DV, HID)

    in_maps = []
    for core in range(NC_N):
        hs = [2 * core, 2 * core + 1]
        nopes = np.concatenate([wqf[:, h, :DN] for h in hs], 1)
        pes = []
        for h in hs:
            pe = wqf[:, h, DN:]
            pes += [pe[:, 0::2], pe[:, 1::2]]
        wq_c = np.concatenate([nopes] + pes, 1)
        wkvb_c = np.concatenate(
            [wkvbf[:, hs[0], :DN], wkvbf[:, hs[1], :DN],
             wkvbf[:, hs[0], DN:], wkvbf[:, hs[1], DN:]], 1)   # [512, 512]
        in_maps.append({
            "htb": htb,
            "hto": np.ascontiguousarray(
                hTt[:, :, core * TO:(core + 1) * TO]).astype(BF),
            "wq": tilemaj(wq_c, HCH),
            "wkva": tilemaj(wkva_p, HCH),
            "wkvb": tilemaj(wkvb_c, KC),
            "wo": np.ascontiguousarray(
                np.concatenate([wof[h] for h in hs], 0).reshape(
                    HPC, P, HID).transpose(1, 0, 2)).astype(BF),
            "wg": tilemaj(wgf[:, core * FPC:(core + 1) * FPC], HCH),
            "wu": tilemaj(wuf[:, core * FPC:(core + 1) * FPC], HCH),
            "wd": tilemaj(wdf[core * FPC:(core + 1) * FPC, :], FPC // P),
            "cosf": cosf.astype(BF),
            "sinf": sinf.astype(BF),
            "masks": masks,
        })
    return in_maps


def kernel(**inputs):
    if "nc" not in _CACHE:
        _CACHE["nc"] = _build()
    nc = _CACHE["nc"]
    in_maps = _prep(**inputs)
    res = run_bass_kernel_spmd(nc, in_maps, core_ids=list(range(NC_N)))
    xT = np.empty((HCH, NC_N, 16, T), np.float32)
    for c in range(NC_N):
        slab = np.asarray(res.results[c]["o"], dtype=np.float32)
        slab = slab.reshape(16, TT, HCH, TW)          # [p, t, o, col]
        xT[:, c, :, :] = slab.transpose(2, 0, 1, 3).reshape(HCH, 16, T)
    return np.ascontiguousarray(
        xT.reshape(HID, T).T).reshape(B, S, HID)



# revision 21
# speedup vs baseline: 1.0871x; 1.0871x over previous
"""DeepseekV2 decoder layer on 8 TRN2 NeuronCores (Bass/Tile).

Sharding: TP over heads (2/core) for q/kv_b/attention/o_proj, kv_a sharded
over tokens (256/core) + AllGather (fp8), TP over INTER (1024/core) for the
MLP.  Chunked RS+AG after o_proj and chunked ReduceScatter after down_proj,
overlapped with compute.

Numerics: the attention-side projections (q, kv_a, kv_b, o_proj) run as
fp8-e4m3 DoubleRow matmuls (2x PE throughput).  Static weights are
host-prepped into the SwInterleave layout (column-reversed, k-slot pairs
byte-interleaved); moving operands are plain [P, 2, N] fp8 slot slices.
Weights are pre-scaled by powers of two (256/32/64) to dodge the fp8
subnormal floor; the inverse scales ride existing activation-scale/bias
params and psum-copy multiplies.  Scores, PV, and the whole MLP stay bf16
(fp8 there would blow the 2e-2 error budget).

Cross-partition broadcasts and the softmax-denominator reduction run on
the (otherwise idle) GpSimd engine via partition_broadcast /
partition_all_reduce instead of ones-vector matmuls, keeping the tensor
queue free of reduction stalls.  o_proj is software-pipelined one chunk
deep so each chunk's softmax tail hides under the next chunk's scores.
"""

import numpy as np
import ml_dtypes

import concourse.bass as bass
import concourse.bass_isa as bass_isa
import concourse.mybir as mybir
import concourse.tile as tile
from concourse import bacc
from concourse.bass_utils import run_bass_kernel_spmd

BF = ml_dtypes.bfloat16
E4 = ml_dtypes.float8_e4m3fn

B, S, HID = 2, 1024, 2048
T = B * S                      # 2048 tokens
H = 16
DN, DR = 128, 64
DQK = DN + DR
DV = 128
KVR = 512
INTER = 8192
EPS = 1e-6
ROPE_BASE = 10000.0
SCALING = DQK ** -0.5

NC_N = 8
HPC = H // NC_N                # 2 heads per core
FPC = INTER // NC_N            # 1024 inter per core
P = 128
HCH = HID // P                 # 16 hid chunks
HG = HCH // 2                  # 8 hid pair-groups (fp8 DoubleRow)
TT = 4                         # token chunks of 512
TW = T // TT                   # 512
TO = T // NC_N                 # 256 own tokens for kv_a
KT = S // P                    # 8 k-tiles of 128 per batch
QT = S // TW                   # 2 q-chunks of 512 per batch
KC = KVR // P                  # 4 kv-lora chunks
NEG = -30000.0

SQ = 256.0                     # wq / wkva fp8 scale
SKB = 32.0                     # wkv_b fp8 scale
SO = 64.0                      # wo fp8 scale

f32 = mybir.dt.float32
bf16 = mybir.dt.bfloat16
fp8 = mybir.dt.float8e4
ADD = mybir.AluOpType.add
MUL = mybir.AluOpType.mult
BYP = mybir.AluOpType.bypass
AF = mybir.ActivationFunctionType
SWI = mybir.MatmulPerfMode.DoubleRowSwInterleave
RADD = bass_isa.ReduceOp.add

_CACHE = {}


def _build():
    nc = bacc.Bacc("TRN2", target_bir_lowering=False, debug=False, num_devices=NC_N)
    dp = lambda n, sh, dt: nc.dram_tensor(n, sh, dt, kind="ExternalInput")
    ht8d = dp("ht8", [P, TT, HCH, TW], fp8)      # hidden^T fp8, chunk-tiled
    htbd = dp("htb", [P, TT, HCH, TW], bf16)     # hidden^T bf16 (residual)
    hto8d = dp("hto8", [P, HCH, TO], fp8)        # own-token slice of hidden^T
    wq = dp("wq", [P, HG, 3, P, 2], fp8)         # swil [h0n,h1n,pe]
    wkva = dp("wkva", [P, HG, KVR + P, 2], fp8)   # swil: 4x128 kv + pe padded to 128
    wkvbn = dp("wkvbn", [P, KC // 2, HPC, P, 2], fp8)   # swil k_nope part
    wkvbv = dp("wkvbv", [P, KC, HPC * DV], fp8)  # v part (moving side)
    wo = dp("wo", [P, HCH, P, 2], fp8)           # swil o_proj (one k-pair)
    wg = dp("wg", [P, HCH, FPC], bf16)
    wu = dp("wu", [P, HCH, FPC], bf16)
    wd = dp("wd", [P, FPC // P, HID], bf16)
    cosf = dp("cosf", [P, T], bf16)
    sinf = dp("sinf", [P, T], bf16)
    masks = dp("masks", [P, 4, TW], bf16)
    out = nc.dram_tensor("o", [16, TT * HCH * TW], bf16, kind="ExternalOutput")
    rg = [list(range(NC_N))]

    with tile.TileContext(nc) as tc:
        with tc.tile_pool(name="const", bufs=1) as cpool, \
             tc.tile_pool(name="dram", bufs=1, space="DRAM") as dram, \
             tc.tile_pool(name="mlpw", bufs=1) as mlpw, \
             tc.tile_pool(name="bpre", bufs=1) as bpre:
            ones_col = cpool.tile([P, 1], bf16)
            nc.vector.memset(ones_col[:], 1.0)
            epsr = cpool.tile([1, 1], f32)
            nc.vector.memset(epsr[:], EPS)
            epsp = cpool.tile([P, 1], f32)
            nc.vector.memset(epsp[:], EPS)
            nlsq = cpool.tile([1, 1], f32)
            nc.vector.memset(nlsq[:], -float(np.log(SQ)))

            ag_in = dram.tile([P, KC * TO], fp8, name="ag_in")
            ag_out = dram.tile([NC_N * P, KC * TO], fp8, addr_space="Shared",
                               name="ag_out")
            # o_proj reduction as RS + AG (4x less wire than mesh AllReduce)
            ar_in = [dram.tile([P, HCH, TW], bf16, name=f"ar_in{t}")
                     for t in range(TT)]
            o_rs = [dram.tile([16, HCH * TW], bf16, name=f"o_rs{t}")
                    for t in range(TT)]
            o_ag = [dram.tile([P, HCH, TW], bf16, addr_space="Shared",
                              name=f"o_ag{t}") for t in range(TT)]
            # chunks 0-2 reduce-scatter 2MB; chunk 3 in four 0.5MB
            # quarters to shrink the tail
            rs_in = [dram.tile([P, HCH, TW], bf16, name=f"rs_in{t}")
                     for t in range(TT - 1)]
            rs_in += [dram.tile([P, HCH // 4, TW], bf16, name=f"rs_in3{i}")
                      for i in range(4)]
            rs_out = [dram.tile([16, HCH * TW], bf16, name=f"rs_out{t}")
                      for t in range(TT - 1)]
            rs_out += [dram.tile([16, HCH // 4 * TW], bf16, name=f"rs_out3{i}")
                       for i in range(4)]

            # phase-B chunk loads (issued early, scalar DMA ring; a waiting
            # descriptor blocks only that ring, not scalar compute)
            xts, arts = [None] * TT, [None] * TT

            def load_x(t):
                xt = bpre.tile([P, HCH, TW], bf16, name=f"x{t}", tag="x",
                               bufs=2)
                nc.scalar.dma_start(xt[:], htbd.ap()[:, t, :, :])
                xts[t] = xt

            def load_ar(t):
                art = bpre.tile([P, HCH, TW], bf16, name=f"ar{t}", tag="ar",
                                bufs=1)
                nc.scalar.dma_start(art[:], o_ag[t][:])
                arts[t] = art

            with tc.tile_pool(name="keep", bufs=1) as keep:
                qsb = keep.tile([P, 3, T], bf16)           # 12K
                kpe2 = keep.tile([P, T], bf16)             # 4K
                kva2 = keep.tile([P, NC_N, KC, TO], fp8, name="kva2")   # 8K
                wkvbn_sb = keep.tile([P, KC // 2, HPC, P, 2], fp8,
                                     name="wkvbn_sb")
                wkvbv_sb = keep.tile([P, KC, HPC * DV], fp8, name="wkvbv_sb")
                wo_sb = keep.tile([P, HCH, P, 2], fp8, name="wo_sb")

                # ============ Phase A1: kv_a first, then q proj ============
                with tc.tile_pool(name="a1", bufs=1) as a1, \
                     tc.tile_pool(name="awrk", bufs=2) as awrk, \
                     tc.tile_pool(name="arow", bufs=2) as arow, \
                     tc.tile_pool(name="aps", bufs=1, space="PSUM") as aps:
                    hto_sb = a1.tile([P, HCH, TO], fp8)
                    nc.scalar.dma_start(hto_sb[:], hto8d.ap())
                    wkva_sb = a1.tile([P, HG, KVR + P, 2], fp8)
                    nc.scalar.dma_start(wkva_sb[:], wkva.ap())
                    wq_sb = a1.tile([P, HG, 3, P, 2], fp8)
                    nc.sync.dma_start(wq_sb[:], wq.ap())
                    nc.gpsimd.dma_start(wkvbn_sb[:], wkvbn.ap())
                    nc.gpsimd.dma_start(wkvbv_sb[:], wkvbv.ap())
                    nc.gpsimd.dma_start(wo_sb[:], wo.ap())

                    # -- kv_a for OWN 256 tokens, AllGather immediately --
                    lat_own = a1.tile([P, KC, TO], fp8)
                    lps = []
                    for f in range(KC):
                        lp = aps.tile([P, TO], f32, tag="lp", bufs=4, name="lp")
                        for g in range(HG):
                            nc.tensor.matmul(lp[:],
                                             wkva_sb[:, g, f * P:(f + 1) * P, :],
                                             hto_sb[:, 2 * g:2 * g + 2, :],
                                             start=(g == 0), stop=(g == HG - 1),
                                             perf_mode=SWI)
                        lps.append(lp)
                    # squares of the raw (x256) latent; 256^2 cancels inside
                    # rsqrt so bc2 lands on normalized-latent scale directly
                    sql = []
                    for f in range(KC):
                        sq = awrk.tile([P, TO], bf16, tag="sql", bufs=4,
                                       name="sql")
                        nc.scalar.square(sq[:], lps[f][:])
                        sql.append(sq)
                    nc.vector.tensor_tensor(sql[0][:], sql[0][:], sql[1][:], ADD)
                    nc.vector.tensor_tensor(sql[2][:], sql[2][:], sql[3][:], ADD)
                    nc.vector.tensor_tensor(sql[0][:], sql[0][:], sql[2][:], ADD)
                    ss2p = aps.tile([1, TO], f32, tag="ss", bufs=2, name="ss2p")
                    nc.tensor.matmul(ss2p[:], ones_col[:], sql[0][:],
                                     start=True, stop=True)
                    lg2 = arow.tile([1, TO], f32, tag="srow", name="lg2")
                    nc.scalar.activation(lg2[:], ss2p[:], AF.Ln,
                                         bias=epsr[:], scale=1.0 / KVR)
                    rb2 = arow.tile([1, TO], bf16, tag="rb", name="rb2")
                    nc.scalar.activation(rb2[:], lg2[:], AF.Exp, scale=-0.5)
                    bc2 = a1.tile([P, TO], bf16, name="bc2")
                    nc.gpsimd.partition_broadcast(bc2[:], rb2[:])
                    for f in range(KC):
                        nc.vector.tensor_tensor(lat_own[:, f, :], lps[f][:],
                                                bc2[:], MUL)
                    nc.sync.dma_start(ag_in[:], lat_own[:])
                    nc.gpsimd.collective_compute(
                        "AllGather", BYP, ins=[ag_in[:].opt()],
                        outs=[ag_out[:].opt()], replica_groups=rg)

                    # -- q/k_pe proj per chunk; rmsnorm scales batched per
                    # batch to avoid act-table thrash and tensor stalls --
                    srows = [None] * TT
                    bc1s = [None] * TT

                    def a1_batch_tail(b):
                        # batched Ln/Exp (one table each) + bcast + scales
                        for t in (2 * b, 2 * b + 1):
                            lg = arow.tile([1, TW], f32, tag="lgrow", name="lg")
                            nc.scalar.activation(lg[:], srows[t][:], AF.Ln,
                                                 bias=epsr[:], scale=1.0 / HID)
                            rb = arow.tile([1, TW], bf16, tag="rb", name="rb")
                            # exp(-0.5*ln(ms) - ln(256)) = rsqrt(ms)/256:
                            # folds away the x256 fp8 scale of wq/wkva
                            nc.scalar.activation(rb[:], lg[:], AF.Exp,
                                                 scale=-0.5, bias=nlsq[:])
                            bc1 = a1.tile([P, TW], bf16, tag="bc1", bufs=2,
                                          name="bc1")
                            nc.gpsimd.partition_broadcast(bc1[:], rb[:])
                            bc1s[t] = bc1
                        for t in (2 * b, 2 * b + 1):
                            bc1 = bc1s[t]
                            for f in range(3):
                                nc.vector.tensor_tensor(
                                    qsb[:, f, t * TW:(t + 1) * TW],
                                    qsb[:, f, t * TW:(t + 1) * TW], bc1[:],
                                    MUL)
                            nc.vector.tensor_tensor(
                                kpe2[:DR, t * TW:(t + 1) * TW],
                                kpe2[:DR, t * TW:(t + 1) * TW], bc1[:DR, :],
                                MUL)
                            nc.sync.dma_start(kpe2[DR:, t * TW:(t + 1) * TW],
                                              kpe2[:DR, t * TW:(t + 1) * TW])

                    for t in range(TT):
                        ht_t = a1.tile([P, HCH, TW], fp8, tag="ht", bufs=2,
                                       name="ht_t")
                        nc.sync.dma_start(ht_t[:], ht8d.ap()[:, t, :, :])
                        # 8 pairs: one square on scalar + one on vector + add
                        sqps = []
                        for i in range(8):
                            sqa = awrk.tile([P, TW], bf16, tag="sqa", bufs=2,
                                            name="sqa")
                            nc.scalar.square(sqa[:], ht_t[:, 2 * i, :])
                            sqp = awrk.tile([P, TW], bf16, tag="sqp", bufs=8,
                                            name="sqp")
                            nc.vector.tensor_tensor(sqp[:], ht_t[:, 2 * i + 1, :],
                                                    ht_t[:, 2 * i + 1, :], MUL)
                            nc.vector.tensor_tensor(sqp[:], sqp[:], sqa[:],
                                                    ADD)
                            sqps.append(sqp)

                        # q + k_pe projections (fp8 DoubleRow), scalar-Copy
                        # psum drains; ssq ones-matmuls go AFTER the q chains
                        # in the tensor queue so squares never stall them
                        for f in range(3):
                            qp = aps.tile([P, TW], f32, tag="big", bufs=2,
                                          name="qp")
                            for g in range(HG):
                                nc.tensor.matmul(qp[:], wq_sb[:, g, f, :, :],
                                                 ht_t[:, 2 * g:2 * g + 2, :],
                                                 start=(g == 0),
                                                 stop=(g == HG - 1),
                                                 perf_mode=SWI)
                            nc.scalar.activation(qsb[:, f, t * TW:(t + 1) * TW],
                                                 qp[:], AF.Copy)
                        kp2 = aps.tile([P, TW], f32, tag="big", bufs=2,
                                       name="kp2")
                        for g in range(HG):
                            nc.tensor.matmul(kp2[:],
                                             wkva_sb[:, g, KVR:KVR + P, :],
                                             ht_t[:, 2 * g:2 * g + 2, :],
                                             start=(g == 0), stop=(g == HG - 1),
                                             perf_mode=SWI)
                        nc.scalar.activation(kpe2[:DR, t * TW:(t + 1) * TW],
                                             kp2[:DR, :], AF.Copy)
                        ssp = aps.tile([1, TW], f32, tag="ss", bufs=2,
                                       name="ssp")
                        for i in range(8):
                            nc.tensor.matmul(ssp[:], ones_col[:], sqps[i][:],
                                             start=(i == 0), stop=(i == 7))
                        srow = arow.tile([1, TW], f32, tag="srow", bufs=2,
                                         name="srow")
                        nc.vector.tensor_copy(out=srow[:], in_=ssp[:])
                        srows[t] = srow
                        if t % 2 == 1:
                            a1_batch_tail(t // 2)

                    # readback of gathered kv_a
                    for r in range(NC_N):
                        nc.gpsimd.dma_start(kva2[:, r, :, :],
                                            ag_out[r * P:(r + 1) * P, :])

                # ---- A2: kv_b, rope, attention, o_proj (+RS/AG) ----
                with tc.tile_pool(name="a2", bufs=1) as a2, \
                     tc.tile_pool(name="awrk2", bufs=2) as awrk2, \
                     tc.tile_pool(name="arow2", bufs=2) as arow2, \
                     tc.tile_pool(name="aps2", bufs=1, space="PSUM") as aps2:
                    cs = a2.tile([P, T], bf16)
                    nc.scalar.dma_start(cs[:], cosf.ap())
                    sn = a2.tile([P, T], bf16)
                    nc.scalar.dma_start(sn[:], sinf.ap())
                    msk = a2.tile([P, 4, TW], bf16)
                    nc.scalar.dma_start(msk[:], masks.ap())
                    # prefetch the big MLP weights early (consumed in phase B)
                    wg_sb = mlpw.tile([P, HCH, FPC], bf16)       # 32K
                    nc.scalar.dma_start(wg_sb[:], wg.ap())
                    wu_sb = mlpw.tile([P, HCH, FPC], bf16)       # 32K
                    nc.scalar.dma_start(wu_sb[:], wu.ap())

                    # kv_b: k_nope (fp8 DoubleRow) + v (fp8 x fp8 plain)
                    knope = a2.tile([P, HPC, T], bf16)
                    for h in range(HPC):
                        for t2 in range(NC_N):
                            kp = aps2.tile([P, TO], f32, tag="big", bufs=2,
                                           name="kp")
                            for cg in range(KC // 2):
                                nc.tensor.matmul(
                                    kp[:], wkvbn_sb[:, cg, h, :, :],
                                    kva2[:, t2, 2 * cg:2 * cg + 2, :],
                                    start=(cg == 0), stop=(cg == KC // 2 - 1),
                                    perf_mode=SWI)
                            # x(1/32) removes the fp8 weight scale
                            nc.scalar.activation(
                                knope[:, h, t2 * TO:(t2 + 1) * TO], kp[:],
                                AF.Copy, scale=1.0 / SKB)
                    vnat = a2.tile([P, T // P, HPC * DV], bf16)
                    for to in range(T // P):
                        vp = aps2.tile([P, HPC * DV], f32, tag="vp", bufs=1,
                                       name="vp")
                        for c in range(KC):
                            nc.tensor.matmul(vp[:],
                                             kva2[:, to // 2, c,
                                                  (to % 2) * P:(to % 2 + 1) * P],
                                             wkvbv_sb[:, c, :],
                                             start=(c == 0), stop=(c == KC - 1))
                        nc.vector.tensor_scalar_mul(vnat[:, to, :], vp[:],
                                                    1.0 / SKB)

                    # rope in place, one batch at a time (lets batch-0
                    # scores start before batch-1 rope):
                    # qsb[:,2,:] rows are [h0x1,h0x2,h1x1,h1x2], kpe2 rows
                    # are [x1,x2,x1,x2]; cs=[c,c,c,c], sn=[-s,s,-s,s]
                    def rope_batch(b):
                        c0, c1 = b * S, (b + 1) * S
                        for src_ap in (qsb[:, 2, c0:c1], kpe2[:, c0:c1]):
                            swp = a2.tile([P, S], bf16, tag="swp", bufs=2,
                                          name="swp")
                            for g in range(4):
                                half = 32 if g % 2 == 0 else -32
                                nc.sync.dma_start(
                                    swp[g * 32:(g + 1) * 32, :],
                                    src_ap[g * 32 + half:(g + 1) * 32 + half, :])
                            rtmp = a2.tile([P, S], bf16, tag="rtmp", bufs=2,
                                           name="rtmp")
                            nc.vector.tensor_tensor(rtmp[:], src_ap,
                                                    cs[:, c0:c1], MUL)
                            nc.vector.tensor_tensor(src_ap, swp[:],
                                                    sn[:, c0:c1], MUL)
                            nc.vector.tensor_tensor(src_ap, src_ap, rtmp[:],
                                                    ADD)

                    rope_batch(0)
                    rope_batch(1)

                    # attention: scores/PV bf16;                    # attention: scores/PV bf16; denominators via gpsimd
                    # all-reduce; o_proj fp8 DoubleRow, software-pipelined
                    attns = {}
                    pend = None

                    def flush_oproj(tt):
                        attn_t = attns[tt]
                        for ho in range(HCH):
                            op = aps2.tile([P, TW], f32, tag="big", bufs=2,
                                           name="op")
                            nc.tensor.matmul(op[:], wo_sb[:, ho, :, :],
                                             attn_t[:, 0:2, :],
                                             start=True, stop=True,
                                             perf_mode=SWI)
                            # x(1/64) removes the fp8 wo scale
                            oall = a2.tile([P, TW], bf16, tag="oall", bufs=2,
                                           name="oall")
                            nc.vector.tensor_scalar_mul(oall[:], op[:],
                                                        1.0 / SO)
                            nc.sync.dma_start(ar_in[tt][:, ho, :], oall[:])
                        nc.gpsimd.collective_compute(
                            "ReduceScatter", ADD, ins=[ar_in[tt][:].opt()],
                            outs=[o_rs[tt][:].opt()], replica_groups=rg)
                        nc.gpsimd.collective_compute(
                            "AllGather", BYP, ins=[o_rs[tt][:].opt()],
                            outs=[o_ag[tt][:].opt()], replica_groups=rg)

                    for b in range(B):
                        for qt in range(QT):
                            tt = b * QT + qt
                            qc0 = b * S + qt * TW
                            nkt = 4 * qt + 4
                            attn_t = a2.tile([P, HPC, TW], fp8, tag="attn",
                                             bufs=2, name="attn_t")
                            attns[tt] = attn_t
                            daccs, atps = [], []
                            for h in range(HPC):
                                atp = aps2.tile([P, TW], f32, tag="att",
                                                bufs=3, name="atp")
                                dacc = awrk2.tile([P, TW], bf16, tag="dacc",
                                                  bufs=4, name="dacc")
                                atps.append(atp)
                                daccs.append(dacc)
                                exs = [None] * nkt

                                def consume(kt):
                                    nc.tensor.matmul(atp[:],
                                                     vnat[:, b * KT + kt,
                                                          h * DV:(h + 1) * DV],
                                                     exs[kt][:],
                                                     start=(kt == 0),
                                                     stop=(kt == nkt - 1))

                                for kt in range(nkt):
                                    kc0 = b * S + kt * P
                                    scp = aps2.tile([P, TW], f32, tag="big",
                                                    bufs=2, name="scp")
                                    nc.tensor.matmul(scp[:],
                                                     knope[:, h, kc0:kc0 + P],
                                                     qsb[:, h, qc0:qc0 + TW],
                                                     start=True, stop=False)
                                    nc.tensor.matmul(
                                        scp[:],
                                        kpe2[h * DR:(h + 1) * DR, kc0:kc0 + P],
                                        qsb[h * DR:(h + 1) * DR, 2,
                                            qc0:qc0 + TW],
                                        start=False, stop=True)
                                    ex = awrk2.tile([P, TW], bf16, tag="ex",
                                                    bufs=4, name="ex")
                                    j = kt - 4 * qt
                                    if j >= 0:
                                        mtmp = awrk2.tile([P, TW], f32,
                                                          tag="mt", name="mtmp")
                                        nc.vector.tensor_tensor(
                                            mtmp[:], scp[:], msk[:, j, :], ADD)
                                        nc.scalar.activation(ex[:], mtmp[:],
                                                             AF.Exp)
                                    else:
                                        nc.scalar.activation(ex[:], scp[:],
                                                             AF.Exp)
                                    exs[kt] = ex
                                    if kt == 1:
                                        nc.vector.tensor_tensor(
                                            dacc[:], exs[0][:], exs[1][:], ADD)
                                    elif kt > 1:
                                        nc.vector.tensor_tensor(dacc[:],
                                                                dacc[:],
                                                                ex[:], ADD)
                                    if kt >= 2:
                                        consume(kt - 2)
                                consume(max(nkt - 2, 0))
                                if nkt > 1:
                                    consume(nkt - 1)
                            # denominators: ones-matmul row sum (cheap, after
                            # both heads' scores so it never stalls), fast
                            # reciprocal on vector, broadcast on gpsimd
                            dnps = []
                            for h in range(HPC):
                                dnp = aps2.tile([1, TW], f32, tag="den",
                                                bufs=1, name="dnp")
                                nc.tensor.matmul(dnp[:], ones_col[:],
                                                 daccs[h][:],
                                                 start=True, stop=True)
                                dnps.append(dnp)
                            if pend is not None:
                                flush_oproj(pend)
                                if pend == 0:
                                    load_x(0)
                                    load_ar(0)
                                elif pend == 1:
                                    load_x(1)
                            pend = tt
                            for h in range(HPC):
                                rcp = arow2.tile([1, TW], f32, tag="rcp",
                                                 bufs=2, name="rcp")
                                nc.vector.reciprocal_approx_fast(rcp[:],
                                                                 dnps[h][:])
                                rec = awrk2.tile([P, TW], f32, tag="rec",
                                                 bufs=2, name="rec")
                                nc.gpsimd.partition_broadcast(rec[:], rcp[:])
                                nc.vector.tensor_tensor(attn_t[:, h, :],
                                                        atps[h][:], rec[:],
                                                        MUL)
                    flush_oproj(pend)

            # ============ Phase B: residual + norm + MLP ============
            with tc.tile_pool(name="bbig", bufs=1) as bbig, \
                 tc.tile_pool(name="bwrk", bufs=2) as bwrk, \
                 tc.tile_pool(name="bps", bufs=1, space="PSUM") as bps:

                wd_sb = bbig.tile([P, FPC // P, HID], bf16)  # 32K
                nc.scalar.dma_start(wd_sb[:], wd.ap())

                for t in range(TT):
                    # x = hidden + attn_out; later x/8 in place
                    x, arall = xts[t], arts[t]
                    sq3s = []
                    for g in range(4):
                        sqg3 = bwrk.tile([P, TW], bf16, tag="sqg3", bufs=4,
                                         name="sqg3")
                        for k in range(4):
                            o = 4 * g + k
                            nc.vector.tensor_tensor(x[:, o, :], x[:, o, :],
                                                    arall[:, o, :], ADD)
                            dst = sqg3
                            if k != 0:
                                dst = bwrk.tile([P, TW], bf16, tag="sq3",
                                                bufs=2, name="sq3")
                            if (o % 2) == 0:
                                nc.scalar.square(dst[:], x[:, o, :])
                            else:
                                nc.vector.tensor_tensor(dst[:], x[:, o, :],
                                                        x[:, o, :], MUL)
                            if k != 0:
                                nc.vector.tensor_tensor(sqg3[:], sqg3[:],
                                                        dst[:], ADD)
                        sq3s.append(sqg3)
                    nc.vector.tensor_tensor(sq3s[0][:], sq3s[0][:],
                                            sq3s[1][:], ADD)
                    nc.vector.tensor_tensor(sq3s[2][:], sq3s[2][:],
                                            sq3s[3][:], ADD)
                    nc.vector.tensor_tensor(sq3s[0][:], sq3s[0][:],
                                            sq3s[2][:], ADD)
                    # next chunk's attn-output and next-next residual loads
                    if t + 1 < TT:
                        load_ar(t + 1)
                    if t + 2 < TT:
                        load_x(t + 2)
                    ssa = bwrk.tile([P, TW], f32, tag="ssa", bufs=2, name="ssa")
                    nc.gpsimd.partition_all_reduce(ssa[:], sq3s[0][:], P, RADD)
                    lg3 = bwrk.tile([P, TW], f32, tag="lg3", bufs=2, name="lg3")
                    nc.scalar.activation(lg3[:], ssa[:], AF.Ln,
                                         bias=epsp[:], scale=1.0 / HID)
                    bc3 = bwrk.tile([P, TW], bf16, tag="bc3", bufs=2,
                                    name="bc3")
                    nc.scalar.activation(bc3[:], lg3[:], AF.Exp, scale=-0.5)
                    h2 = bbig.tile([P, HCH, TW], bf16, name="h2", tag="h2",
                                   bufs=1)
                    for o in range(HCH):
                        nc.vector.tensor_tensor(h2[:, o, :], x[:, o, :],
                                                bc3[:], MUL)
                    # x -> x/8 in place (folded residual for ReduceScatter)
                    nc.vector.tensor_scalar_mul(x[:, :, :], x[:, :, :], 0.125)

                    # gate/up/silu
                    act = bbig.tile([P, FPC // P, TW], bf16, name="act",
                                    tag="act", bufs=1)
                    for fi in range(FPC // P):
                        gp = bps.tile([P, TW], f32, tag="gu", bufs=4, name="gp")
                        for o in range(HCH):
                            nc.tensor.matmul(gp[:],
                                             wg_sb[:, o, fi * P:(fi + 1) * P],
                                             h2[:, o, :],
                                             start=(o == 0),
                                             stop=(o == HCH - 1))
                        up = bps.tile([P, TW], f32, tag="gu", bufs=4, name="up")
                        for o in range(HCH):
                            nc.tensor.matmul(up[:],
                                             wu_sb[:, o, fi * P:(fi + 1) * P],
                                             h2[:, o, :],
                                             start=(o == 0),
                                             stop=(o == HCH - 1))
                        gs = bwrk.tile([P, TW], f32, tag="gs", bufs=2,
                                       name="gs")
                        nc.scalar.activation(gs[:], gp[:], AF.Silu)
                        nc.vector.tensor_tensor(act[:, fi, :], up[:], gs[:],
                                                MUL)

                    # down projection partial (+x/8) + RS
                    # last chunk reduces in two 1MB halves to shrink the tail
                    dall2 = bbig.tile([P, HCH, TW], bf16, name="dall2",
                                      tag="dall2", bufs=1)
                    for ho in range(HCH):
                        dpp = bps.tile([P, TW], f32, tag="d", bufs=2,
                                       name="dpp")
                        for c in range(FPC // P):
                            nc.tensor.matmul(dpp[:],
                                             wd_sb[:, c, ho * P:(ho + 1) * P],
                                             act[:, c, :],
                                             start=(c == 0),
                                             stop=(c == FPC // P - 1))
                        nc.vector.tensor_tensor(dall2[:, ho, :], dpp[:],
                                                x[:, ho, :], ADD)
                        if t == TT - 1 and ho % 4 == 3:
                            q = ho // 4
                            HQ = HCH // 4
                            nc.sync.dma_start(rs_in[3 + q][:],
                                              dall2[:, q * HQ:(q + 1) * HQ, :])
                            nc.gpsimd.collective_compute(
                                "ReduceScatter", ADD,
                                ins=[rs_in[3 + q][:].opt()],
                                outs=[rs_out[3 + q][:].opt()],
                                replica_groups=rg)
                            nc.sync.dma_start(
                                out.ap()[:, (3 * HCH + q * HQ) * TW:
                                         (3 * HCH + (q + 1) * HQ) * TW],
                                rs_out[3 + q][:])
                    if t < TT - 1:
                        nc.sync.dma_start(rs_in[t][:], dall2[:])
                        nc.gpsimd.collective_compute(
                            "ReduceScatter", ADD, ins=[rs_in[t][:].opt()],
                            outs=[rs_out[t][:].opt()], replica_groups=rg)
                        nc.sync.dma_start(
                            out.ap()[:, t * HCH * TW:(t + 1) * HCH * TW],
                            rs_out[t][:])
    nc.compile()
    return nc


def _swil(w, blocks):
    """Host prep: SwInterleave layout for fp8 DoubleRow stationary weights.

    w: [K, M] float32 (already scaled), K % 256 == 0.  ``blocks`` lists the
    column widths of the individual matmul blocks (reversal is per block).
    Returns [P, K//256, M, 2] e4m3: per k-pair group, per block, columns
    reversed with the two k-slots byte-interleaved.
    """
    K, M = w.shape
    G = K // 256
    wr = w.reshape(G, 2, P, M)
    out = np.empty((P, G, M, 2), np.float32)
    c0 = 0
    for blk in blocks:
        sl = slice(c0, c0 + blk)
        rev = slice(c0 + blk - 1, c0 - 1 if c0 > 0 else None, -1)
        for s in range(2):
            out[:, :, sl, s] = wr[:, s, :, rev].transpose(1, 0, 2)
        c0 += blk
    assert c0 == M
    return np.clip(out, -240.0, 240.0).astype(E4)


def _prep(hidden_states, positions, w_in_ln, w_q, w_kv_a, w_kv_a_ln,
          w_kv_b, w_o, w_post_ln, w_gate, w_up, w_down):
    hT = np.ascontiguousarray(
        np.asarray(hidden_states, np.float32).reshape(T, HID).T)
    hTt = hT.reshape(HCH, P, T).transpose(1, 0, 2)          # [128, 16, T]
    htb_f = np.ascontiguousarray(
        hTt.reshape(P, HCH, TT, TW).transpose(0, 2, 1, 3))
    htb = htb_f.astype(BF)
    ht8 = np.clip(htb_f, -240, 240).astype(E4)

    pos = np.asarray(positions).reshape(-1).astype(np.float64)
    inv = ROPE_BASE ** (-np.arange(0, DR, 2, dtype=np.float64) / DR)
    fr = pos[:, None] * inv[None, :]                      # [T, 32]
    c32 = np.cos(fr).T.astype(np.float32)                 # [32, T]
    s32 = np.sin(fr).T.astype(np.float32)
    cosf = np.concatenate([c32] * 4, 0)
    sinf = np.concatenate([-s32, s32, -s32, s32], 0)

    r = np.arange(P)[:, None]
    c = np.arange(TW)[None, :]
    masks = np.stack([np.where(c >= r + j * P, 0.0, NEG) for j in range(4)],
                     1).astype(np.float32)                # [128, 4, 512]

    w_in_ln = np.asarray(w_in_ln, np.float32)
    wqf = (np.asarray(w_q, np.float32) * w_in_ln[:, None] * SCALING
           ).reshape(HID, H, DQK)
    wkvaf = np.asarray(w_kv_a, np.float32) * w_in_ln[:, None]
    kpe_w = wkvaf[:, KVR:]
    pe_pair = np.concatenate([kpe_w[:, 0::2], kpe_w[:, 1::2]], 1)
    wkva_p = np.concatenate([wkvaf[:, :KVR], pe_pair,
                             np.zeros((HID, P - DR), np.float32)], 1)
    wkvbf = (np.asarray(w_kv_b, np.float32)
             * np.asarray(w_kv_a_ln, np.float32)[:, None]).reshape(KVR, H,
                                                                   DN + DV)
    w_post_ln = np.asarray(w_post_ln, np.float32)
    wgf = np.asarray(w_gate, np.float32) * w_post_ln[:, None]
    wuf = np.asarray(w_up, np.float32) * w_post_ln[:, None]
    wdf = np.asarray(w_down, np.float32)
    wof = np.asarray(w_o, np.float32).reshape(H, DV, HID)

    def tilemaj(a, chunks):
        # [rows, cols] -> [128, chunks, cols], partition-major
        return np.ascontiguousarray(
            a.reshape(chunks, P, -1).transpose(1, 0, 2)).astype(BF)

    in_maps = []
    for core in range(NC_N):
        hs = [2 * core, 2 * core + 1]
        nopes = np.concatenate([wqf[:, h, :DN] for h in hs], 1)
        pes = []
        for h in hs:
            pe = wqf[:, h, DN:]
            pes += [pe[:, 0::2], pe[:, 1::2]]
        wq_c = np.concatenate([nopes] + pes, 1)              # [HID, 384]
        wq8 = _swil(wq_c * SQ, [P, P, P]).reshape(P, HG, 3, P, 2)
        wkva8 = _swil(wkva_p * SQ, [P, P, P, P, P])
        wkvbn_c = np.concatenate(
            [wkvbf[:, hs[0], :DN], wkvbf[:, hs[1], :DN]], 1)  # [512, 256]
        wkvbn8 = _swil(wkvbn_c * SKB, [P, P]).reshape(P, KC // 2, HPC, P, 2)
        wkvbv_c = np.concatenate(
            [wkvbf[:, hs[0], DN:], wkvbf[:, hs[1], DN:]], 1)  # [512, 256]
        wkvbv8 = np.clip(
            wkvbv_c.reshape(KC, P, HPC * DV).transpose(1, 0, 2) * SKB,
            -240, 240).astype(E4)
        wo_c = np.concatenate([wof[h] for h in hs], 0)        # [256, HID]
        wo8 = _swil(wo_c * SO, [P] * HCH).reshape(P, HCH, P, 2)

        in_maps.append({
            "ht8": ht8,
            "htb": htb,
            "hto8": np.clip(np.ascontiguousarray(
                hTt[:, :, core * TO:(core + 1) * TO]), -240, 240).astype(E4),
            "wq": wq8,
            "wkva": wkva8,
            "wkvbn": wkvbn8,
            "wkvbv": wkvbv8,
            "wo": wo8,
            "wg": tilemaj(wgf[:, core * FPC:(core + 1) * FPC], HCH),
            "wu": tilemaj(wuf[:, core * FPC:(core + 1) * FPC], HCH),
            "wd": tilemaj(wdf[core * FPC:(core + 1) * FPC, :], FPC // P),
            "cosf": cosf.astype(BF),
            "sinf": sinf.astype(BF),
            "masks": masks.astype(BF),
        })
    return in_maps


def kernel(**inputs):
    if "nc" not in _CACHE:
        _CACHE["nc"] = _build()
    nc = _CACHE["nc"]
    in_maps = _prep(**inputs)
    res = run_bass_kernel_spmd(nc, in_maps, core_ids=list(range(NC_N)))
    xT = np.empty((HCH, NC_N, 16, T), np.float32)
    for c in range(NC_N):
        slab = np.asarray(res.results[c]["o"], dtype=np.float32)
        slab = slab.reshape(16, TT, HCH, TW)          # [p, t, o, col]
        xT[:, c, :, :] = slab.transpose(2, 0, 1, 3).reshape(HCH, 16, T)
    return np.ascontiguousarray(
        xT.reshape(HID, T).T).reshape(B, S, HID)
